# revision 1
# baseline (speedup 1.0000x reference)
"""VQ codebook lookup kernel for Trainium2 (8 NeuronCores, data-parallel).

Computes out[b] = values[argmin_k ||x[b] - keys[k]||] for
x [65536, 512], keys/values [1024, 512] fp32.

Strategy (per core, batch shard of 8192 rows):
  - argmin of distance == argmax of s = 2*x.k - |k|^2 (sqrt and the
    |x|^2 row offset do not change the argmin).
  - near-fp32 matmul precision at bf16 speed via hi/lo split: three
    bf16 passes (hi*hi + hi*lo + lo*hi) accumulated in PSUM; the
    dropped lo*lo term is ~1e-4 absolute vs typical top-2 margins ~5.
  - Host prep (layout only): transpose the x shard to [512, 8192],
    split x and (2*keys)^T into bf16 hi/lo, precompute |k|^2.
  - Device per 128-row tile: 24 PE matmuls (N=512, K=128) -> DVE
    subtract of |k|^2 fused with the PSUM->SBUF move -> DVE
    MAX8/FIND_INDEX8 for per-row argmax -> indirect-DMA gather of
    values rows -> DMA out.
"""

import numpy as np

_B = 65536
_D = 512
_K = 1024
_NCORES = 8
_BL = _B // _NCORES  # 8192 rows per core
_P = 128
_BBLK = 512          # b columns loaded per DMA
_BT = 128            # b rows per matmul tile (PSUM partition dim)
_DC = _D // _P       # 4 contraction chunks

_cached = None


def _build():
    import concourse.mybir as mybir
    from concourse import bacc
    from concourse.bass import IndirectOffsetOnAxis
    from concourse.tile import TileContext

    f32 = mybir.dt.float32
    bf16 = mybir.dt.bfloat16
    u32 = mybir.dt.uint32

    nc = bacc.Bacc("TRN2", target_bir_lowering=False, debug=False,
                   num_devices=_NCORES)
    # x hi/lo packed as one tensor: rows 0..511 = bf16 hi, 512..1023 = bf16 lo
    xTb = nc.dram_tensor("xTb", [2 * _D, _BL], bf16, kind="ExternalInput")
    kTh = nc.dram_tensor("kTh", [_D, _K], bf16, kind="ExternalInput")
    kTl = nc.dram_tensor("kTl", [_D, _K], bf16, kind="ExternalInput")
    k2r = nc.dram_tensor("k2r", [_P, _K], f32, kind="ExternalInput")
    vals = nc.dram_tensor("vals", [_K, _D], f32, kind="ExternalInput")
    out = nc.dram_tensor("out", [_BL, _D], f32, kind="ExternalOutput")

    xTb3 = xTb.rearrange("(do p) b -> p do b", p=_P)   # [128, 8, 8192]
    kTh3 = kTh.rearrange("(do p) k -> p do k", p=_P)   # [128, 4, 1024]
    kTl3 = kTl.rearrange("(do p) k -> p do k", p=_P)

    with TileContext(nc) as tc:
        with (
            tc.tile_pool(name="const", bufs=1) as cpool,
            tc.tile_pool(name="xp", bufs=3) as xpool,
            tc.tile_pool(name="warm", bufs=1) as warmpool,
            tc.tile_pool(name="sp", bufs=3) as spool,
            tc.tile_pool(name="st", bufs=4) as stpool,
            tc.tile_pool(name="gp", bufs=4) as gpool,
            tc.tile_pool(name="ps", bufs=3, space="PSUM") as pspool,
            tc.tile_pool(name="wps", bufs=1, space="PSUM") as wpspool,
        ):
            # Const loads go on the Scalar engine's HWDGE queue so they
            # overlap with the x-block loads issued from the Sync engine
            # (descriptor generation serializes per issuing engine).
            # Ordered by when tile 0 consumes them: kh half-0 first.
            kh_sb = cpool.tile([_P, _DC, _K], bf16)
            kl_sb = cpool.tile([_P, _DC, _K], bf16)
            k2_sb = cpool.tile([_P, _K], f32)
            nc.scalar.dma_start(kh_sb[:, :, 0:512], kTh3[:, :, 0:512])
            nc.scalar.dma_start(kl_sb[:, :, 0:512], kTl3[:, :, 0:512])
            nc.scalar.dma_start(kh_sb[:, :, 512:1024], kTh3[:, :, 512:1024])
            nc.scalar.dma_start(kl_sb[:, :, 512:1024], kTl3[:, :, 512:1024])
            nc.scalar.dma_start(k2_sb[:], k2r[:, :])

            # Pre-warm the PE clock (HAM) during the initial DMA wait:
            # ~4us of dummy matmuls on memset scratch lifts the PE from
            # 1.2GHz to 2.4GHz before the real stream begins.
            wsrc = warmpool.tile([_P, 64], bf16)
            nc.vector.memset(wsrc[:], 0.0)
            wps = wpspool.tile([_P, 64], f32)
            for _ in range(72):
                nc.tensor.matmul(wps[:64, :], lhsT=wsrc[:, :64], rhs=wsrc[:],
                                 start=True, stop=True)

            # First block is a single b-tile so the PE starts sooner;
            # remaining blocks are _BBLK wide.
            blocks = [(0, _BT)]
            off = _BT
            while off < _BL:
                w = min(_BBLK, _BL - off)
                blocks.append((off, w))
                off += w

            for boff, bw in blocks:
                xtb = xpool.tile([_P, 2 * _DC, _BBLK], bf16, tag="xtb")
                nc.sync.dma_start(xtb[:, :, :bw], xTb3[:, :, boff:boff + bw])

                for sub in range(bw // _BT):
                    bt = boff // _BT + sub
                    bsl = slice(sub * _BT, (sub + 1) * _BT)
                    ps = pspool.tile([_P, _K], f32)
                    s = spool.tile([_P, _K], f32)
                    for h in range(2):
                        hsl = slice(h * 512, (h + 1) * 512)
                        po = ps[:, hsl]
                        # kh-only passes first so tile 0 need not wait for
                        # the kl const load (it lands behind kh at startup)
                        for dc in range(_DC):
                            nc.tensor.matmul(po, lhsT=xtb[:, dc, bsl],
                                             rhs=kh_sb[:, dc, hsl],
                                             start=(dc == 0), stop=False)
                            nc.tensor.matmul(po, lhsT=xtb[:, _DC + dc, bsl],
                                             rhs=kh_sb[:, dc, hsl],
                                             start=False, stop=False)
                        for dc in range(_DC):
                            nc.tensor.matmul(po, lhsT=xtb[:, dc, bsl],
                                             rhs=kl_sb[:, dc, hsl],
                                             start=False, stop=(dc == _DC - 1))
                        # s = 2*x.k - |k|^2, fused PSUM->SBUF move
                        nc.vector.tensor_sub(
                            out=s[:, hsl], in0=po, in1=k2_sb[:, hsl])
                    mx = stpool.tile([_P, 8], f32)
                    nc.vector.max(out=mx[:], in_=s[:])
                    idx = stpool.tile([_P, 8], u32)
                    nc.vector.max_index(out=idx[:], in_max=mx[:], in_values=s[:])

                    g = gpool.tile([_P, _D], f32)
                    nc.gpsimd.indirect_dma_start(
                        out=g[:],
                        out_offset=None,
                        in_=vals[:, :],
                        in_offset=IndirectOffsetOnAxis(ap=idx[:, :1], axis=0),
                    )
                    nc.scalar.dma_start(out[bt * _BT:(bt + 1) * _BT, :], g[:])

    nc.compile()
    return nc


def _get_nc():
    global _cached
    if _cached is None:
        _cached = _build()
    return _cached


def _hi_lo(a):
    """Split fp32 array into bf16 hi + bf16 lo with hi + lo ~ a."""
    import ml_dtypes

    hi = a.astype(ml_dtypes.bfloat16)
    lo = (a - hi.astype(np.float32)).astype(ml_dtypes.bfloat16)
    return hi, lo


def _prepare_in_maps(x, keys, values):
    x = np.asarray(x, dtype=np.float32)
    keys = np.asarray(keys, dtype=np.float32)
    values = np.asarray(values, dtype=np.float32)

    k2T = np.ascontiguousarray((2.0 * keys).T)          # [512, 1024] f32
    kTh, kTl = _hi_lo(k2T)

    k2 = np.einsum("kd,kd->k", keys, keys).astype(np.float32)
    k2r = np.ascontiguousarray(np.broadcast_to(k2, (_P, _K)))

    in_maps = []
    for c in range(_NCORES):
        xs = np.ascontiguousarray(x[c * _BL:(c + 1) * _BL].T)  # [512, 8192]
        xh, xl = _hi_lo(xs)
        xb = np.concatenate([xh, xl], axis=0)                  # [1024, 8192]
        in_maps.append({"xTb": xb, "kTh": kTh, "kTl": kTl,
                        "k2r": k2r, "vals": values})
    return in_maps


def kernel(x, keys, values):
    from concourse.bass_utils import run_bass_kernel_spmd

    nc = _get_nc()
    in_maps = _prepare_in_maps(x, keys, values)
    res = run_bass_kernel_spmd(nc, in_maps, core_ids=list(range(_NCORES)))
    return np.concatenate([r["out"] for r in res.results], axis=0)



# revision 3
# speedup vs baseline: 1.2563x; 1.2563x over previous
"""VQ codebook lookup kernel for Trainium2 (8 NeuronCores, data-parallel).

Computes out[b] = values[argmin_k ||x[b] - keys[k]||] for
x [65536, 512], keys/values [1024, 512] fp32.

Strategy (per core, batch shard of 8192 rows):
  - argmin of distance == argmax of s = 2*x.k - |k|^2 (sqrt and the
    |x|^2 row offset do not change the argmin).
  - Precision via fp16 hi pass + fp8 DoubleRow correction:
      s ~= xh.Kh  +  (xh.Kl + xl.Kh)
    where xh = fp16(x), xl = x - xh (and likewise for K = 2*keys).
    The hi pass runs in fp16 (full PE rate, 11-bit significand); the
    two cross terms run as ONE fp8-e4m3 DoubleRow pass with 1024-deep
    contraction (2x contraction per instruction).  Host simulation of
    this exact quantization gives 0/65536 argmax mismatches.
  - All operands carry power-of-2 scales so every matmul product lands
    at 2^18 * (term): hi pass (2^9 xh)x(2^9 Kh); DR pairs
    (2^5 xh)x(2^13 Kl) and (2^15 xl)x(2^3 Kh).  All 8 MMs accumulate
    into one PSUM bank, so post-processing is a single fused DVE
    subtract of 2^18*|k|^2 (scaling does not change the argmax).
  - Device per 128-row tile: 8 fp16 MMs + 8 fp8-DR MMs -> DVE subtract
    fused with PSUM->SBUF -> DVE MAX8/FIND_INDEX8 argmax ->
    indirect-DMA gather of values rows -> DMA out.
"""

import numpy as np

_B = 65536
_D = 512
_K = 1024
_NCORES = 8
_BL = _B // _NCORES  # 8192 rows per core
_P = 128
_BBLK = 512          # b columns loaded per DMA
_BT = 128            # b rows per matmul tile (PSUM partition dim)
_DC = _D // _P       # 4 contraction chunks

_cached = None


def _build():
    import concourse.mybir as mybir
    from concourse import bacc
    from concourse.bass import IndirectOffsetOnAxis
    from concourse.tile import TileContext

    f32 = mybir.dt.float32
    f16 = mybir.dt.float16
    f8 = mybir.dt.float8e4
    u32 = mybir.dt.uint32
    DR = mybir.MatmulPerfMode.DoubleRow

    nc = bacc.Bacc("TRN2", target_bir_lowering=False, debug=False,
                   num_devices=_NCORES)
    xf = nc.dram_tensor("xf", [_D, _BL], f16, kind="ExternalInput")
    xq = nc.dram_tensor("xq", [2 * _D, _BL], f8, kind="ExternalInput")
    kf = nc.dram_tensor("kf", [_D, _K], f16, kind="ExternalInput")
    kq = nc.dram_tensor("kq", [2 * _D, _K], f8, kind="ExternalInput")
    k2r = nc.dram_tensor("k2r", [_P, _K], f32, kind="ExternalInput")
    vals = nc.dram_tensor("vals", [_K, _D], f32, kind="ExternalInput")
    out = nc.dram_tensor("out", [_BL, _D], f32, kind="ExternalOutput")

    xf3 = xf.rearrange("(do p) b -> p do b", p=_P)     # [128, 4, 8192]
    xq3 = xq.rearrange("(do p) b -> p do b", p=_P)     # [128, 8, 8192]
    kf3 = kf.rearrange("(do p) k -> p do k", p=_P)     # [128, 4, 1024]
    kq3 = kq.rearrange("(do p) k -> p do k", p=_P)     # [128, 8, 1024]

    with TileContext(nc) as tc:
        with (
            tc.tile_pool(name="const", bufs=1) as cpool,
            tc.tile_pool(name="xp", bufs=3) as xpool,
            tc.tile_pool(name="warm", bufs=1) as warmpool,
            tc.tile_pool(name="sp", bufs=3) as spool,
            tc.tile_pool(name="st", bufs=4) as stpool,
            tc.tile_pool(name="gp", bufs=4) as gpool,
            tc.tile_pool(name="ps", bufs=3, space="PSUM") as pspool,
            tc.tile_pool(name="wps", bufs=1, space="PSUM") as wpspool,
        ):
            # Const loads go on the Scalar engine's HWDGE queue so they
            # overlap with the x-block loads issued from the Sync engine
            # (descriptor generation serializes per issuing engine).
            # Ordered by when tile 0 consumes them: kf half-0 first.
            kf_sb = cpool.tile([_P, _DC, _K], f16)
            kq_sb = cpool.tile([_P, 2 * _DC, _K], f8)
            k2_sb = cpool.tile([_P, _K], f32)
            nc.scalar.dma_start(kf_sb[:, :, 0:512], kf3[:, :, 0:512])
            nc.scalar.dma_start(kq_sb[:, :, 0:512], kq3[:, :, 0:512])
            nc.scalar.dma_start(kf_sb[:, :, 512:1024], kf3[:, :, 512:1024])
            nc.scalar.dma_start(kq_sb[:, :, 512:1024], kq3[:, :, 512:1024])
            nc.scalar.dma_start(k2_sb[:], k2r[:, :])

            # Pre-warm the PE clock (HAM) during the initial DMA wait:
            # ~4us of dummy matmuls on memset scratch lifts the PE from
            # 1.2GHz to 2.4GHz before the real stream begins.
            wsrc = warmpool.tile([_P, 64], f16)
            nc.vector.memset(wsrc[:], 0.0)
            wps = wpspool.tile([_P, 64], f32)
            for _ in range(72):
                nc.tensor.matmul(wps[:64, :], lhsT=wsrc[:, :64], rhs=wsrc[:],
                                 start=True, stop=True)

            # First block is a single b-tile so the PE starts sooner;
            # remaining blocks are _BBLK wide.
            blocks = [(0, _BT)]
            off = _BT
            while off < _BL:
                w = min(_BBLK, _BL - off)
                blocks.append((off, w))
                off += w

            for boff, bw in blocks:
                xft = xpool.tile([_P, _DC, _BBLK], f16, tag="xft")
                xqt = xpool.tile([_P, 2 * _DC, _BBLK], f8, tag="xqt")
                nc.sync.dma_start(xft[:, :, :bw], xf3[:, :, boff:boff + bw])
                nc.sync.dma_start(xqt[:, :, :bw], xq3[:, :, boff:boff + bw])

                for sub in range(bw // _BT):
                    bt = boff // _BT + sub
                    bsl = slice(sub * _BT, (sub + 1) * _BT)
                    ps = pspool.tile([_P, _K], f32)
                    s = spool.tile([_P, _K], f32)
                    for h in range(2):
                        hsl = slice(h * 512, (h + 1) * 512)
                        po = ps[:, hsl]
                        # fp16 hi pass: 2^18 * xh.Kh
                        for dc in range(_DC):
                            nc.tensor.matmul(po, lhsT=xft[:, dc, bsl],
                                             rhs=kf_sb[:, dc, hsl],
                                             start=(dc == 0), stop=False)
                        # fp8 DoubleRow correction: 2^18*(xh.Kl + xl.Kh),
                        # 256-deep contraction per instruction.
                        for c in range(_DC):
                            csl = slice(2 * c, 2 * c + 2)
                            nc.tensor.matmul(po, lhsT=xqt[:, csl, bsl],
                                             rhs=kq_sb[:, csl, hsl],
                                             start=False, stop=(c == _DC - 1),
                                             perf_mode=DR)
                        # s = 2^18*(2*x.k - |k|^2), fused PSUM->SBUF move
                        nc.vector.tensor_sub(
                            out=s[:, hsl], in0=po, in1=k2_sb[:, hsl])
                    mx = stpool.tile([_P, 8], f32)
                    nc.vector.max(out=mx[:], in_=s[:])
                    idx = stpool.tile([_P, 8], u32)
                    nc.vector.max_index(out=idx[:], in_max=mx[:], in_values=s[:])

                    g = gpool.tile([_P, _D], f32)
                    nc.gpsimd.indirect_dma_start(
                        out=g[:],
                        out_offset=None,
                        in_=vals[:, :],
                        in_offset=IndirectOffsetOnAxis(ap=idx[:, :1], axis=0),
                    )
                    nc.scalar.dma_start(out[bt * _BT:(bt + 1) * _BT, :], g[:])

    nc.compile()
    return nc


def _get_nc():
    global _cached
    if _cached is None:
        _cached = _build()
    return _cached


def _prepare_in_maps(x, keys, values):
    import ml_dtypes

    f8 = ml_dtypes.float8_e4m3

    x = np.asarray(x, dtype=np.float32)
    keys = np.asarray(keys, dtype=np.float32)
    values = np.asarray(values, dtype=np.float32)

    K2T = np.ascontiguousarray((2.0 * keys).T)          # [512, 1024] f32
    Kh16 = K2T.astype(np.float16)
    Kh = Kh16.astype(np.float32)
    Kl = K2T - Kh
    kf = (Kh * 2.0**9).astype(np.float16)               # exact pow2 scale
    kq = np.concatenate([
        np.clip(Kl * 2.0**13, -240, 240).astype(f8),    # pairs with xh chunks
        np.clip(Kh * 2.0**3, -240, 240).astype(f8),     # pairs with xl chunks
    ], axis=0)                                          # [1024, 1024]

    k2 = np.einsum("kd,kd->k", keys.astype(np.float64),
                   keys.astype(np.float64))
    k2r = np.ascontiguousarray(
        np.broadcast_to((k2 * 2.0**18).astype(np.float32), (_P, _K)))

    in_maps = []
    for c in range(_NCORES):
        xs = np.ascontiguousarray(x[c * _BL:(c + 1) * _BL].T)  # [512, 8192]
        xh16 = xs.astype(np.float16)
        xh = xh16.astype(np.float32)
        xl = xs - xh
        xf = (xh * 2.0**9).astype(np.float16)
        xq = np.concatenate([
            np.clip(xh * 2.0**5, -240, 240).astype(f8),
            np.clip(xl * 2.0**15, -240, 240).astype(f8),
        ], axis=0)                                      # [1024, 8192]
        in_maps.append({"xf": xf, "xq": xq, "kf": kf, "kq": kq,
                        "k2r": k2r, "vals": values})
    return in_maps


def kernel(x, keys, values):
    from concourse.bass_utils import run_bass_kernel_spmd

    nc = _get_nc()
    in_maps = _prepare_in_maps(x, keys, values)
    res = run_bass_kernel_spmd(nc, in_maps, core_ids=list(range(_NCORES)))
    return np.concatenate([r["out"] for r in res.results], axis=0)


# revision 12
# speedup vs baseline: 1.3980x; 1.1128x over previous
"""VQ codebook lookup kernel for Trainium2 (8 NeuronCores, data-parallel).

Computes out[b] = values[argmin_k ||x[b] - keys[k]||] for
x [65536, 512], keys/values [1024, 512] fp32.

Strategy (per core, batch shard of 8192 rows):
  - argmin of distance == argmax of s = 2*x.k - |k|^2 (sqrt and the
    |x|^2 row offset do not change the argmin).
  - Precision via fp16 hi pass + fp8 DoubleRow correction:
      s ~= xh.Kh  +  (xh.Kl + xl.Kh)
    where xh = fp16(x), xl = x - xh (and likewise for K = 2*keys).
    The hi pass runs in fp16 (full PE rate, 11-bit significand); the
    two cross terms run as ONE fp8-e4m3 DoubleRow pass with 1024-deep
    contraction (2x contraction per instruction).  Host simulation of
    this exact quantization gives 0/65536 argmax mismatches.
  - All operands carry power-of-2 scales so every matmul product lands
    at 2^18 * (term): hi pass (2^9 xh)x(2^9 Kh); DR pairs
    (2^5 xh)x(2^13 Kl) and (2^15 xl)x(2^3 Kh).  All 16 MMs of a
    128-row tile accumulate into one 2-bank PSUM tile (DR groups
    first, so the accumulation 'stop' lands on a cheap fp16 MM).
  - Post-matmul per tile: DVE subtract of 2^18*|k|^2 fused with the
    PSUM->SBUF move, DVE MAX8/FIND_INDEX8 argmax -> indirect-DMA
    gather of values rows -> DMA out.
"""

import numpy as np

_B = 65536
_D = 512
_K = 1024
_NCORES = 8
_BL = _B // _NCORES  # 8192 rows per core
_P = 128
_BBLK = 512          # b columns loaded per DMA
_BT = 128            # b rows per matmul tile (PSUM partition dim)
_DC = _D // _P       # 4 contraction chunks

_cached = None


def _build():
    import concourse.mybir as mybir
    from concourse import bacc
    from concourse.bass import IndirectOffsetOnAxis
    from concourse.tile import TileContext

    f32 = mybir.dt.float32
    f16 = mybir.dt.float16
    f8 = mybir.dt.float8e4
    u32 = mybir.dt.uint32
    DR = mybir.MatmulPerfMode.DoubleRow

    nc = bacc.Bacc("TRN2", target_bir_lowering=False, debug=False,
                   num_devices=_NCORES)
    xf = nc.dram_tensor("xf", [_D, _BL], f16, kind="ExternalInput")
    xq = nc.dram_tensor("xq", [2 * _D, _BL], f8, kind="ExternalInput")
    kf = nc.dram_tensor("kf", [_D, _K], f16, kind="ExternalInput")
    kq = nc.dram_tensor("kq", [2 * _D, _K], f8, kind="ExternalInput")
    k2r = nc.dram_tensor("k2r", [_P, _K], f32, kind="ExternalInput")
    vals = nc.dram_tensor("vals", [_K, _D], f32, kind="ExternalInput")
    out = nc.dram_tensor("out", [_BL, _D], f32, kind="ExternalOutput")

    xf3 = xf.rearrange("(do p) b -> p do b", p=_P)     # [128, 4, 8192]
    xq3 = xq.rearrange("(do p) b -> p do b", p=_P)     # [128, 8, 8192]
    kf3 = kf.rearrange("(do p) k -> p do k", p=_P)     # [128, 4, 1024]
    kq3 = kq.rearrange("(do p) k -> p do k", p=_P)     # [128, 8, 1024]

    with TileContext(nc) as tc:
        with (
            tc.tile_pool(name="const", bufs=1) as cpool,
            tc.tile_pool(name="xp", bufs=3) as xpool,
            tc.tile_pool(name="warm", bufs=1) as warmpool,
            tc.tile_pool(name="sp", bufs=3) as spool,
            tc.tile_pool(name="st", bufs=4) as stpool,
            tc.tile_pool(name="gp", bufs=4) as gpool,
            tc.tile_pool(name="ps", bufs=3, space="PSUM") as pspool,
            tc.tile_pool(name="wps", bufs=1, space="PSUM") as wpspool,
        ):
            # Const loads go on the Scalar engine's HWDGE queue so they
            # overlap with the x-block loads issued from the Sync engine.
            # DR groups run first, so kq loads first.
            kf_sb = cpool.tile([_P, _DC, _K], f16)
            kq_sb = cpool.tile([_P, 2 * _DC, _K], f8)
            k2_sb = cpool.tile([_P, _K], f32)
            nc.scalar.dma_start(kq_sb[:], kq3[:, :, :])
            nc.scalar.dma_start(kf_sb[:], kf3[:, :, :])
            nc.scalar.dma_start(k2_sb[:], k2r[:, :])

            # Pre-warm the PE clock (HAM) during the initial DMA wait:
            # ~4us of dummy matmuls on memset scratch lifts the PE from
            # 1.2GHz to 2.4GHz before the real stream begins.
            wsrc = warmpool.tile([_P, 64], f16)
            nc.vector.memset(wsrc[:], 0.0)
            wps = wpspool.tile([_P, 64], f32)
            for _ in range(72):
                nc.tensor.matmul(wps[:64, :], lhsT=wsrc[:, :64], rhs=wsrc[:],
                                 start=True, stop=True)

            # First block is a single b-tile so the PE starts sooner;
            # remaining blocks are _BBLK wide.
            blocks = [(0, _BT)]
            off = _BT
            while off < _BL:
                w = min(_BBLK, _BL - off)
                blocks.append((off, w))
                off += w

            H0 = slice(0, 512)
            H1 = slice(512, 1024)

            for boff, bw in blocks:
                xft = xpool.tile([_P, _DC, _BBLK], f16, tag="xft")
                xqt = xpool.tile([_P, 2 * _DC, _BBLK], f8, tag="xqt")
                nc.sync.dma_start(xqt[:, :, :bw], xq3[:, :, boff:boff + bw])
                nc.sync.dma_start(xft[:, :, :bw], xf3[:, :, boff:boff + bw])

                for sub in range(bw // _BT):
                    bt = boff // _BT + sub
                    bsl = slice(sub * _BT, (sub + 1) * _BT)
                    ps = pspool.tile([_P, _K], f32)
                    poA, poB = ps[:, H0], ps[:, H1]
                    # fp8 DoubleRow correction first (h0/h1 interleaved
                    # so each weight load serves two matmuls):
                    # 2^18*(xh.Kl + xl.Kh), 256-deep contraction per MM.
                    for c in range(_DC):
                        csl = slice(2 * c, 2 * c + 2)
                        nc.tensor.matmul(poA, lhsT=xqt[:, csl, bsl],
                                         rhs=kq_sb[:, csl, H0],
                                         start=(c == 0), stop=False,
                                         perf_mode=DR)
                        nc.tensor.matmul(poB, lhsT=xqt[:, csl, bsl],
                                         rhs=kq_sb[:, csl, H1],
                                         start=(c == 0), stop=False,
                                         perf_mode=DR)
                    # fp16 hi pass: 2^18 * xh.Kh ('stop' lands here)
                    for dc in range(_DC):
                        nc.tensor.matmul(poA, lhsT=xft[:, dc, bsl],
                                         rhs=kf_sb[:, dc, H0],
                                         start=False, stop=(dc == _DC - 1))
                        nc.tensor.matmul(poB, lhsT=xft[:, dc, bsl],
                                         rhs=kf_sb[:, dc, H1],
                                         start=False, stop=(dc == _DC - 1))

                    # s = 2^18*(2x.k - |k|^2), fused PSUM->SBUF move
                    s = spool.tile([_P, _K], f32)
                    nc.vector.tensor_sub(out=s[:, H0], in0=poA,
                                         in1=k2_sb[:, H0])
                    nc.vector.tensor_sub(out=s[:, H1], in0=poB,
                                         in1=k2_sb[:, H1])
                    mx = stpool.tile([_P, 8], f32)
                    nc.vector.max(out=mx[:], in_=s[:])
                    idx = stpool.tile([_P, 8], u32)
                    nc.vector.max_index(out=idx[:], in_max=mx[:], in_values=s[:])

                    g = gpool.tile([_P, _D], f32)
                    nc.gpsimd.indirect_dma_start(
                        out=g[:],
                        out_offset=None,
                        in_=vals[:, :],
                        in_offset=IndirectOffsetOnAxis(ap=idx[:, :1], axis=0),
                    )
                    nc.scalar.dma_start(out[bt * _BT:(bt + 1) * _BT, :], g[:])

    nc.compile()
    return nc


def _get_nc():
    global _cached
    if _cached is None:
        _cached = _build()
    return _cached


def _prepare_in_maps(x, keys, values):
    import ml_dtypes

    f8 = ml_dtypes.float8_e4m3

    x = np.asarray(x, dtype=np.float32)
    keys = np.asarray(keys, dtype=np.float32)
    values = np.asarray(values, dtype=np.float32)

    K2T = np.ascontiguousarray((2.0 * keys).T)          # [512, 1024] f32
    Kh16 = K2T.astype(np.float16)
    Kh = Kh16.astype(np.float32)
    Kl = K2T - Kh
    kf = (Kh * 2.0**9).astype(np.float16)               # exact pow2 scale
    kq = np.concatenate([
        np.clip(Kl * 2.0**13, -240, 240).astype(f8),    # pairs with xh chunks
        np.clip(Kh * 2.0**3, -240, 240).astype(f8),     # pairs with xl chunks
    ], axis=0)                                          # [1024, 1024]

    k2 = np.einsum("kd,kd->k", keys.astype(np.float64),
                   keys.astype(np.float64))
    k2r = np.ascontiguousarray(
        np.broadcast_to((k2 * 2.0**18).astype(np.float32), (_P, _K)))

    in_maps = []
    for c in range(_NCORES):
        xs = np.ascontiguousarray(x[c * _BL:(c + 1) * _BL].T)  # [512, 8192]
        xh16 = xs.astype(np.float16)
        xh = xh16.astype(np.float32)
        xl = xs - xh
        xf = (xh * 2.0**9).astype(np.float16)
        xq = np.concatenate([
            np.clip(xh * 2.0**5, -240, 240).astype(f8),
            np.clip(xl * 2.0**15, -240, 240).astype(f8),
        ], axis=0)                                      # [1024, 8192]
        in_maps.append({"xf": xf, "xq": xq, "kf": kf, "kq": kq,
                        "k2r": k2r, "vals": values})
    return in_maps


def kernel(x, keys, values):
    from concourse.bass_utils import run_bass_kernel_spmd

    nc = _get_nc()
    in_maps = _prepare_in_maps(x, keys, values)
    res = run_bass_kernel_spmd(nc, in_maps, core_ids=list(range(_NCORES)))
    return np.concatenate([r["out"] for r in res.results], axis=0)


# revision 13
# speedup vs baseline: 1.4032x; 1.0037x over previous
"""VQ codebook lookup kernel for Trainium2 (8 NeuronCores, data-parallel).

Computes out[b] = values[argmin_k ||x[b] - keys[k]||] for
x [65536, 512], keys/values [1024, 512] fp32.

Strategy (per core, batch shard of 8192 rows):
  - argmin of distance == argmax of s = 2*x.k - |k|^2 (sqrt and the
    |x|^2 row offset do not change the argmin).
  - Precision via fp16 hi pass + fp8 DoubleRow correction:
      s ~= xh.Kh  +  (xh.Kl + xl.Kh)
    where xh = fp16(x), xl = x - xh (and likewise for K = 2*keys).
    The hi pass runs in fp16 (full PE rate, 11-bit significand); the
    two cross terms run as ONE fp8-e4m3 DoubleRow pass with 1024-deep
    contraction (2x contraction per instruction).  Host simulation of
    this exact quantization gives 0/65536 argmax mismatches.
  - All operands carry power-of-2 scales so every matmul product lands
    at 2^18 * (term): hi pass (2^9 xh)x(2^9 Kh); DR pairs
    (2^5 xh)x(2^13 Kl) and (2^15 xl)x(2^3 Kh).  All 16 MMs of a
    128-row tile accumulate into one 2-bank PSUM tile (DR groups
    first, so the accumulation 'stop' lands on a cheap fp16 MM).
  - Post-matmul per tile: DVE subtract of 2^18*|k|^2 fused with the
    PSUM->SBUF move, DVE MAX8/FIND_INDEX8 argmax -> indirect-DMA
    gather of values rows -> DMA out.
"""

import numpy as np

_B = 65536
_D = 512
_K = 1024
_NCORES = 8
_BL = _B // _NCORES  # 8192 rows per core
_P = 128
_BBLK = 512          # b columns loaded per DMA
_BT = 128            # b rows per matmul tile (PSUM partition dim)
_DC = _D // _P       # 4 contraction chunks

_cached = None


def _build():
    import concourse.mybir as mybir
    from concourse import bacc
    from concourse.bass import IndirectOffsetOnAxis
    from concourse.tile import TileContext

    f32 = mybir.dt.float32
    f16 = mybir.dt.float16
    f8 = mybir.dt.float8e4
    u32 = mybir.dt.uint32
    DR = mybir.MatmulPerfMode.DoubleRow

    nc = bacc.Bacc("TRN2", target_bir_lowering=False, debug=False,
                   num_devices=_NCORES)
    xf = nc.dram_tensor("xf", [_D, _BL], f16, kind="ExternalInput")
    xq = nc.dram_tensor("xq", [2 * _D, _BL], f8, kind="ExternalInput")
    kf = nc.dram_tensor("kf", [_D, _K], f16, kind="ExternalInput")
    kq = nc.dram_tensor("kq", [2 * _D, _K], f8, kind="ExternalInput")
    k2r = nc.dram_tensor("k2r", [_P, _K], f32, kind="ExternalInput")
    vals = nc.dram_tensor("vals", [_K, _D], f32, kind="ExternalInput")
    out = nc.dram_tensor("out", [_BL, _D], f32, kind="ExternalOutput")

    xf3 = xf.rearrange("(do p) b -> p do b", p=_P)     # [128, 4, 8192]
    xq3 = xq.rearrange("(do p) b -> p do b", p=_P)     # [128, 8, 8192]
    kf3 = kf.rearrange("(do p) k -> p do k", p=_P)     # [128, 4, 1024]
    kq3 = kq.rearrange("(do p) k -> p do k", p=_P)     # [128, 8, 1024]

    with TileContext(nc) as tc:
        with (
            tc.tile_pool(name="const", bufs=1) as cpool,
            tc.tile_pool(name="xp", bufs=3) as xpool,
            tc.tile_pool(name="warm", bufs=1) as warmpool,
            tc.tile_pool(name="sp", bufs=3) as spool,
            tc.tile_pool(name="st", bufs=4) as stpool,
            tc.tile_pool(name="gp", bufs=4) as gpool,
            tc.tile_pool(name="ps", bufs=3, space="PSUM") as pspool,
            tc.tile_pool(name="wps", bufs=1, space="PSUM") as wpspool,
        ):
            # Const loads go on the Scalar engine's HWDGE queue so they
            # overlap with the x-block loads issued from the Sync engine.
            # DR groups run first, so kq loads first.
            kf_sb = cpool.tile([_P, _DC, _K], f16)
            kq_sb = cpool.tile([_P, 2 * _DC, _K], f8)
            k2_sb = cpool.tile([_P, _K], f32)
            nc.scalar.dma_start(kq_sb[:], kq3[:, :, :])
            nc.scalar.dma_start(kf_sb[:], kf3[:, :, :])
            nc.scalar.dma_start(k2_sb[:], k2r[:, :])

            # Pre-warm the PE clock (HAM) during the initial DMA wait:
            # ~4us of dummy matmuls on memset scratch lifts the PE from
            # 1.2GHz to 2.4GHz before the real stream begins.
            wsrc = warmpool.tile([_P, 64], f16)
            nc.vector.memset(wsrc[:], 0.0)
            wps = wpspool.tile([_P, 64], f32)
            for _ in range(72):
                nc.tensor.matmul(wps[:64, :], lhsT=wsrc[:, :64], rhs=wsrc[:],
                                 start=True, stop=True)

            # First block is a single b-tile so the PE starts sooner;
            # remaining blocks are _BBLK wide.
            blocks = [(0, _BT)]
            off = _BT
            while off < _BL:
                w = min(_BBLK, _BL - off)
                blocks.append((off, w))
                off += w

            H0 = slice(0, 512)
            H1 = slice(512, 1024)

            for boff, bw in blocks:
                xft = xpool.tile([_P, _DC, _BBLK], f16, tag="xft")
                xqt = xpool.tile([_P, 2 * _DC, _BBLK], f8, tag="xqt")
                nc.sync.dma_start(xqt[:, :, :bw], xq3[:, :, boff:boff + bw])
                nc.sync.dma_start(xft[:, :, :bw], xf3[:, :, boff:boff + bw])

                for sub in range(bw // _BT):
                    bt = boff // _BT + sub
                    bsl = slice(sub * _BT, (sub + 1) * _BT)
                    ps = pspool.tile([_P, _K], f32)
                    poA, poB = ps[:, H0], ps[:, H1]
                    # fp8 DoubleRow correction first (h0/h1 interleaved
                    # so each weight load serves two matmuls):
                    # 2^18*(xh.Kl + xl.Kh), 256-deep contraction per MM.
                    for c in range(_DC):
                        csl = slice(2 * c, 2 * c + 2)
                        nc.tensor.matmul(poA, lhsT=xqt[:, csl, bsl],
                                         rhs=kq_sb[:, csl, H0],
                                         start=(c == 0), stop=False,
                                         perf_mode=DR)
                        nc.tensor.matmul(poB, lhsT=xqt[:, csl, bsl],
                                         rhs=kq_sb[:, csl, H1],
                                         start=(c == 0), stop=False,
                                         perf_mode=DR)
                    # fp16 hi pass: 2^18 * xh.Kh ('stop' lands here)
                    for dc in range(_DC):
                        nc.tensor.matmul(poA, lhsT=xft[:, dc, bsl],
                                         rhs=kf_sb[:, dc, H0],
                                         start=False, stop=(dc == _DC - 1))
                        nc.tensor.matmul(poB, lhsT=xft[:, dc, bsl],
                                         rhs=kf_sb[:, dc, H1],
                                         start=False, stop=(dc == _DC - 1))

                    # s = 2^18*(2x.k - |k|^2), fused PSUM->SBUF move.
                    # One 1024-wide op spanning both PSUM banks: cheaper
                    # on the (bottleneck) DVE than two 512-wide halves.
                    s = spool.tile([_P, _K], f32)
                    nc.vector.tensor_sub(out=s[:], in0=ps[:], in1=k2_sb[:])
                    mx = stpool.tile([_P, 8], f32)
                    nc.vector.max(out=mx[:], in_=s[:])
                    idx = stpool.tile([_P, 8], u32)
                    nc.vector.max_index(out=idx[:], in_max=mx[:], in_values=s[:])

                    g = gpool.tile([_P, _D], f32)
                    nc.gpsimd.indirect_dma_start(
                        out=g[:],
                        out_offset=None,
                        in_=vals[:, :],
                        in_offset=IndirectOffsetOnAxis(ap=idx[:, :1], axis=0),
                    )
                    nc.scalar.dma_start(out[bt * _BT:(bt + 1) * _BT, :], g[:])

    nc.compile()
    return nc


def _get_nc():
    global _cached
    if _cached is None:
        _cached = _build()
    return _cached


def _prepare_in_maps(x, keys, values):
    import ml_dtypes

    f8 = ml_dtypes.float8_e4m3

    x = np.asarray(x, dtype=np.float32)
    keys = np.asarray(keys, dtype=np.float32)
    values = np.asarray(values, dtype=np.float32)

    K2T = np.ascontiguousarray((2.0 * keys).T)          # [512, 1024] f32
    Kh16 = K2T.astype(np.float16)
    Kh = Kh16.astype(np.float32)
    Kl = K2T - Kh
    kf = (Kh * 2.0**9).astype(np.float16)               # exact pow2 scale
    kq = np.concatenate([
        np.clip(Kl * 2.0**13, -240, 240).astype(f8),    # pairs with xh chunks
        np.clip(Kh * 2.0**3, -240, 240).astype(f8),     # pairs with xl chunks
    ], axis=0)                                          # [1024, 1024]

    k2 = np.einsum("kd,kd->k", keys.astype(np.float64),
                   keys.astype(np.float64))
    k2r = np.ascontiguousarray(
        np.broadcast_to((k2 * 2.0**18).astype(np.float32), (_P, _K)))

    in_maps = []
    for c in range(_NCORES):
        xs = np.ascontiguousarray(x[c * _BL:(c + 1) * _BL].T)  # [512, 8192]
        xh16 = xs.astype(np.float16)
        xh = xh16.astype(np.float32)
        xl = xs - xh
        xf = (xh * 2.0**9).astype(np.float16)
        xq = np.concatenate([
            np.clip(xh * 2.0**5, -240, 240).astype(f8),
            np.clip(xl * 2.0**15, -240, 240).astype(f8),
        ], axis=0)                                      # [1024, 8192]
        in_maps.append({"xf": xf, "xq": xq, "kf": kf, "kq": kq,
                        "k2r": k2r, "vals": values})
    return in_maps


def kernel(x, keys, values):
    from concourse.bass_utils import run_bass_kernel_spmd

    nc = _get_nc()
    in_maps = _prepare_in_maps(x, keys, values)
    res = run_bass_kernel_spmd(nc, in_maps, core_ids=list(range(_NCORES)))
    return np.concatenate([r["out"] for r in res.results], axis=0)


# revision 15
# speedup vs baseline: 1.4477x; 1.0317x over previous
"""VQ codebook lookup kernel for Trainium2 (8 NeuronCores, data-parallel).

Computes out[b] = values[argmin_k ||x[b] - keys[k]||] for
x [65536, 512], keys/values [1024, 512] fp32.

Strategy (per core, batch shard of 8192 rows):
  - argmin of distance == argmax of s = 2*x.k - |k|^2 (sqrt and the
    |x|^2 row offset do not change the argmin).
  - Precision via fp16 hi pass + fp8 DoubleRow correction:
      s ~= xh.Kh  +  (xh.Kl + xl.Kh)
    where xh = fp16(x), xl = x - xh (and likewise for K = 2*keys).
    The hi pass runs in fp16 (full PE rate, 11-bit significand); the
    two cross terms run as ONE fp8-e4m3 DoubleRow pass with 1024-deep
    contraction (2x contraction per instruction).  Host simulation of
    this exact quantization gives 0/65536 argmax mismatches.
  - All operands carry power-of-2 scales so every matmul product lands
    at 2^18 * (term): hi pass (2^9 xh)x(2^9 Kh); DR pairs
    (2^5 xh)x(2^13 Kl) and (2^15 xl)x(2^3 Kh).  All 16 MMs of a
    128-row tile accumulate into one 2-bank PSUM tile (DR groups
    first, so the accumulation 'stop' lands on a cheap fp16 MM).
  - Post-matmul per tile: DVE subtract of 2^18*|k|^2 fused with the
    PSUM->SBUF move, DVE MAX8/FIND_INDEX8 argmax -> indirect-DMA
    gather of values rows -> DMA out.
"""

import numpy as np

_B = 65536
_D = 512
_K = 1024
_NCORES = 8
_BL = _B // _NCORES  # 8192 rows per core
_P = 128
_BBLK = 512          # b columns loaded per DMA
_BT = 128            # b rows per matmul tile (PSUM partition dim)
_DC = _D // _P       # 4 contraction chunks

_cached = None


def _build():
    import concourse.mybir as mybir
    from concourse import bacc
    from concourse.bass import IndirectOffsetOnAxis
    from concourse.tile import TileContext

    f32 = mybir.dt.float32
    f16 = mybir.dt.float16
    f8 = mybir.dt.float8e4
    u32 = mybir.dt.uint32
    DR = mybir.MatmulPerfMode.DoubleRow

    nc = bacc.Bacc("TRN2", target_bir_lowering=False, debug=False,
                   num_devices=_NCORES)
    xf = nc.dram_tensor("xf", [_D, _BL], f16, kind="ExternalInput")
    xq = nc.dram_tensor("xq", [2 * _D, _BL], f8, kind="ExternalInput")
    kf = nc.dram_tensor("kf", [_D, _K], f16, kind="ExternalInput")
    kq = nc.dram_tensor("kq", [2 * _D, _K], f8, kind="ExternalInput")
    k2r = nc.dram_tensor("k2r", [_P, _K], f32, kind="ExternalInput")
    vals = nc.dram_tensor("vals", [_K, _D], f32, kind="ExternalInput")
    out = nc.dram_tensor("out", [_BL, _D], f32, kind="ExternalOutput")

    xf3 = xf.rearrange("(do p) b -> p do b", p=_P)     # [128, 4, 8192]
    xq3 = xq.rearrange("(do p) b -> p do b", p=_P)     # [128, 8, 8192]
    kf3 = kf.rearrange("(do p) k -> p do k", p=_P)     # [128, 4, 1024]
    kq3 = kq.rearrange("(do p) k -> p do k", p=_P)     # [128, 8, 1024]

    with TileContext(nc) as tc:
        with (
            tc.tile_pool(name="const", bufs=1) as cpool,
            tc.tile_pool(name="xp", bufs=3) as xpool,
            tc.tile_pool(name="warm", bufs=1) as warmpool,
            tc.tile_pool(name="sp", bufs=3) as spool,
            tc.tile_pool(name="st", bufs=4) as stpool,
            tc.tile_pool(name="gp", bufs=4) as gpool,
            tc.tile_pool(name="ps", bufs=3, space="PSUM") as pspool,
            tc.tile_pool(name="wps", bufs=1, space="PSUM") as wpspool,
        ):
            # Const loads go on the Scalar engine's HWDGE queue so they
            # overlap with the x-block loads issued from the Sync engine.
            # DR groups run first, so kq loads first.
            kf_sb = cpool.tile([_P, _DC, _K], f16)
            kq_sb = cpool.tile([_P, 2 * _DC, _K], f8)
            k2_sb = cpool.tile([_P, _K], f32)
            nc.scalar.dma_start(kq_sb[:], kq3[:, :, :])
            nc.scalar.dma_start(k2_sb[:], k2r[:, :])
            # kf is issued on the Sync queue right after block-0's x tiles
            # so both 2MB const loads stream in parallel (HWDGE = SP+Act).

            # Pre-warm the PE clock (HAM) during the initial DMA wait:
            # ~4us of dummy matmuls on memset scratch lifts the PE from
            # 1.2GHz to 2.4GHz before the real stream begins.
            wsrc = warmpool.tile([_P, 64], f16)
            nc.vector.memset(wsrc[:], 0.0)
            wps = wpspool.tile([_P, 64], f32)
            for _ in range(72):
                nc.tensor.matmul(wps[:64, :], lhsT=wsrc[:, :64], rhs=wsrc[:],
                                 start=True, stop=True)

            # First block is a single b-tile so the PE starts sooner;
            # remaining blocks are _BBLK wide.
            blocks = [(0, _BT)]
            off = _BT
            while off < _BL:
                w = min(_BBLK, _BL - off)
                blocks.append((off, w))
                off += w

            H0 = slice(0, 512)
            H1 = slice(512, 1024)

            for boff, bw in blocks:
                xft = xpool.tile([_P, _DC, _BBLK], f16, tag="xft")
                xqt = xpool.tile([_P, 2 * _DC, _BBLK], f8, tag="xqt")
                nc.sync.dma_start(xqt[:, :, :bw], xq3[:, :, boff:boff + bw])
                nc.sync.dma_start(xft[:, :, :bw], xf3[:, :, boff:boff + bw])
                if boff == 0:
                    nc.sync.dma_start(kf_sb[:], kf3[:, :, :])

                for sub in range(bw // _BT):
                    bt = boff // _BT + sub
                    bsl = slice(sub * _BT, (sub + 1) * _BT)
                    ps = pspool.tile([_P, _K], f32)
                    poA, poB = ps[:, H0], ps[:, H1]
                    # fp8 DoubleRow correction first (h0/h1 interleaved
                    # so each weight load serves two matmuls):
                    # 2^18*(xh.Kl + xl.Kh), 256-deep contraction per MM.
                    for c in range(_DC):
                        csl = slice(2 * c, 2 * c + 2)
                        nc.tensor.matmul(poA, lhsT=xqt[:, csl, bsl],
                                         rhs=kq_sb[:, csl, H0],
                                         start=(c == 0), stop=False,
                                         perf_mode=DR)
                        nc.tensor.matmul(poB, lhsT=xqt[:, csl, bsl],
                                         rhs=kq_sb[:, csl, H1],
                                         start=(c == 0), stop=False,
                                         perf_mode=DR)
                    # fp16 hi pass: 2^18 * xh.Kh ('stop' lands here)
                    for dc in range(_DC):
                        nc.tensor.matmul(poA, lhsT=xft[:, dc, bsl],
                                         rhs=kf_sb[:, dc, H0],
                                         start=False, stop=(dc == _DC - 1))
                        nc.tensor.matmul(poB, lhsT=xft[:, dc, bsl],
                                         rhs=kf_sb[:, dc, H1],
                                         start=False, stop=(dc == _DC - 1))

                    # s = 2^18*(2x.k - |k|^2), fused PSUM->SBUF move.
                    # One 1024-wide op spanning both PSUM banks: cheaper
                    # on the (bottleneck) DVE than two 512-wide halves.
                    s = spool.tile([_P, _K], f32)
                    nc.vector.tensor_sub(out=s[:], in0=ps[:], in1=k2_sb[:])
                    mx = stpool.tile([_P, 8], f32)
                    nc.vector.max(out=mx[:], in_=s[:])
                    idx = stpool.tile([_P, 8], u32)
                    nc.vector.max_index(out=idx[:], in_max=mx[:], in_values=s[:])

                    g = gpool.tile([_P, _D], f32)
                    nc.gpsimd.indirect_dma_start(
                        out=g[:],
                        out_offset=None,
                        in_=vals[:, :],
                        in_offset=IndirectOffsetOnAxis(ap=idx[:, :1], axis=0),
                    )
                    nc.scalar.dma_start(out[bt * _BT:(bt + 1) * _BT, :], g[:])

    nc.compile()
    return nc


def _get_nc():
    global _cached
    if _cached is None:
        _cached = _build()
    return _cached


def _prepare_in_maps(x, keys, values):
    import ml_dtypes

    f8 = ml_dtypes.float8_e4m3

    x = np.asarray(x, dtype=np.float32)
    keys = np.asarray(keys, dtype=np.float32)
    values = np.asarray(values, dtype=np.float32)

    K2T = np.ascontiguousarray((2.0 * keys).T)          # [512, 1024] f32
    Kh16 = K2T.astype(np.float16)
    Kh = Kh16.astype(np.float32)
    Kl = K2T - Kh
    kf = (Kh * 2.0**9).astype(np.float16)               # exact pow2 scale
    kq = np.concatenate([
        np.clip(Kl * 2.0**13, -240, 240).astype(f8),    # pairs with xh chunks
        np.clip(Kh * 2.0**3, -240, 240).astype(f8),     # pairs with xl chunks
    ], axis=0)                                          # [1024, 1024]

    k2 = np.einsum("kd,kd->k", keys.astype(np.float64),
                   keys.astype(np.float64))
    k2r = np.ascontiguousarray(
        np.broadcast_to((k2 * 2.0**18).astype(np.float32), (_P, _K)))

    in_maps = []
    for c in range(_NCORES):
        xs = np.ascontiguousarray(x[c * _BL:(c + 1) * _BL].T)  # [512, 8192]
        xh16 = xs.astype(np.float16)
        xh = xh16.astype(np.float32)
        xl = xs - xh
        xf = (xh * 2.0**9).astype(np.float16)
        xq = np.concatenate([
            np.clip(xh * 2.0**5, -240, 240).astype(f8),
            np.clip(xl * 2.0**15, -240, 240).astype(f8),
        ], axis=0)                                      # [1024, 8192]
        in_maps.append({"xf": xf, "xq": xq, "kf": kf, "kq": kq,
                        "k2r": k2r, "vals": values})
    return in_maps


def kernel(x, keys, values):
    from concourse.bass_utils import run_bass_kernel_spmd

    nc = _get_nc()
    in_maps = _prepare_in_maps(x, keys, values)
    res = run_bass_kernel_spmd(nc, in_maps, core_ids=list(range(_NCORES)))
    return np.concatenate([r["out"] for r in res.results], axis=0)
